# revision 1
# baseline (speedup 1.0000x reference)
"""Banded additive attention (width-128) on 8 TRN2 NeuronCores — raw Bass.

Problem: B=2, L=2048, F=128, U=32, WIDTH=128
  q = x@Wt, k = x@Wx
  s_ij = Wa . tanh(q_i + k_j + bh) + ba         (j in [i-64, i+63])
  e_ij = exp(sigmoid(s_ij)) * band * mask
  v_i  = sum_j e_ij x_j / (sum_j e_ij + 1e-7)

Sharding: core c handles batch c//4, queries [(c%4)*512, +512).  No
collectives.  Raw Bass (not Tile): the container's walrus rejects >1
semaphore wait per instruction, so all synchronization is explicit
standalone wait_ge instructions with hand-counted thresholds.

Per-core pipeline (partition p = 32*dm + u; d = 4*dg + dm in [0,128)):
  DVE  : Arg[p,(dgl,i)] = q4[p,i] + K4[p,4*dg+i]   (bf16, 4 dgs per sub)
  ACT  : tanh(Arg)                                  (the bulk)
  PE   : S_dg[4,512] = W4.T @ tanh-tile             (u-contraction)
  ACT/DVE: PSUM exit to staging (ACT half fuses sigmoid's tanh)
  DMA  : place staging rows into sg[128=d, 512=i] (engines can't write at
         partition base 4*dg — 32-alignment rule; DMAs can)
  ACT  : finish sigmoid + e = exp(sigmoid) via tanh identity (one table set)
  PE+DVE: circular shear C[c,i] = e[(c-i)%128, i] as 7 log2 stages of
         permutation matmuls on bit-set column sets + merge copies
  GPS  : affine_select triangle masks -> E_lo/E_hi (exact band operands)
  PE   : v[128,132] = E_lo.T @ X[t] + E_hi.T @ X[t+1] (X has a validity
         column so the masked denominator falls out of the same matmul)
  DVE  : v * 1/(den+eps);  DMA out.
"""

import contextlib
import numpy as np
import ml_dtypes

B, L, F, U = 2, 2048, 128, 32
WIDTH = 128
EPS = 1e-7
NCORES = 8
QPC = (B * L) // NCORES          # 512 queries per core
NKEY = QPC + WIDTH               # 640 key rows per core
NSUB = 8                         # dg-subchunks (4 dgs each)
DGS = 4
CH = QPC                         # free width of the score block
BF16 = ml_dtypes.bfloat16

_built = None


def _build():
    import dataclasses
    import concourse.bass as bass
    import concourse.mybir as mybir

    f32 = mybir.dt.float32
    bf16 = mybir.dt.bfloat16
    Tanh = mybir.ActivationFunctionType.Tanh
    Exp = mybir.ActivationFunctionType.Exp

    nc = bass.Bass()

    def rap(ap, newap):
        return dataclasses.replace(ap, ap=newap)

    q4_d = nc.dram_tensor("q4", [128, QPC], bf16, kind="ExternalInput")
    K4_d = nc.dram_tensor("K4", [128, NKEY + 4], bf16, kind="ExternalInput")
    W4_d = nc.dram_tensor("W4", [128, 4], bf16, kind="ExternalInput")
    P7_d = nc.dram_tensor("P7", [128, 128 * 7], bf16, kind="ExternalInput")
    Xe_d = nc.dram_tensor("Xe", [NKEY, 132], bf16, kind="ExternalInput")
    ba_d = nc.dram_tensor("bat", [128, 2], f32, kind="ExternalInput")
    out_d = nc.dram_tensor("out", [QPC, 132], f32, kind="ExternalOutput")


    al = nc.alloc_sbuf_tensor
    q4 = al("q4s", [128, QPC], bf16)
    K4 = al("K4s", [128, NKEY + 4], bf16)
    W4 = al("W4s", [128, 4], bf16)
    P7 = al("P7s", [128, 128 * 7], bf16)
    bat = al("bats", [128, 2], f32)
    X = [al(f"x{t}s", [128, 132], bf16) for t in range(NKEY // 128)]
    arg = [al(f"arg{i}", [128, DGS * CH], bf16) for i in range(2)]
    th = [al(f"th{i}", [128, DGS * CH], bf16) for i in range(2)]
    st32 = al("st32", [4, 32 * CH], f32)
    sg = al("sgs", [128, CH], f32)
    cshear = [al(f"cs{i}", [128, CH], bf16) for i in range(8)]
    El = [al(f"el{t}", [128, 128], bf16) for t in range(4)]
    Eh = [al(f"eh{t}", [128, 128], bf16) for t in range(4)]
    den = [al(f"den{t}", [128, 1], f32) for t in range(4)]
    rden = [al(f"rden{t}", [128, 1], f32) for t in range(4)]
    ov = [al(f"ov{t}", [128, 132], f32) for t in range(4)]

    ap = nc.alloc_psum_tensor
    sp = [ap(f"sp{i}", [4, CH], f32) for i in range(4)]
    rp = [ap(f"rp{i}", [128, CH // 2], f32) for i in range(2)]
    vp = [ap(f"vp{i}", [128, 132], f32) for i in range(2)]

    sem = nc.alloc_semaphore
    sIN, sADD, sTANH, sMM, sXA, sXD = (sem(n) for n in
                                       ("sIN", "sADD", "sTANH", "sMM", "sXA", "sXD"))
    sPL, sEXP, sSMM, sSCP, sMSK, sVMM, sEPI, sOUT = (
        sem(n) for n in ("sPL", "sEXP", "sSMM", "sSCP", "sMSK", "sVMM",
                         "sEPI", "sOUT"))

    AP = bass.AP
    NIN = 16 * 10

    # ---- per-engine streams (raw Bass Block; waits are standalone) ----
    with nc.Block() as block:

        @block.sync
        def _(sync):
            for dst, src in [(q4, q4_d), (K4, K4_d), (W4, W4_d), (P7, P7_d),
                             (bat, ba_d)]:
                sync.dma_start(dst[:, :], src[:, :]).then_inc(sIN, 16)
            for t in range(5):
                sync.dma_start(X[t][:, :],
                               Xe_d[128 * t:128 * (t + 1), :]).then_inc(sIN, 16)
            # placement DMAs: engines cannot write at partition base 4dg
            # (32-alignment rule) but DMAs can.
            for dg in range(32):
                if dg < 8 or 16 <= dg < 24:
                    sync.wait_ge(sXA, (dg + 1) if dg < 8 else (dg - 7))
                else:
                    sync.wait_ge(sXD, (dg - 7) if dg < 16 else (dg - 15))
                sync.dma_start(sg[4 * dg:4 * (dg + 1), :],
                               st32[:, CH * dg:CH * (dg + 1)]).then_inc(sPL, 16)
            for t in range(4):
                sync.wait_ge(sEPI, t + 1)
                sync.dma_start(out_d[128 * t:128 * (t + 1), :],
                               ov[t][:, :]).then_inc(sOUT, 16)

        @block.vector
        def _(vector):
            # adds + DVE psum exits
            order = [("add", 0), ("add", 1), ("add", 2), ("add", 3),
                     ("add", 4)]
            order += [("exit", dg) for dg in range(8, 12)]
            order.append(("add", 5))
            order += [("exit", dg) for dg in range(12, 16)]
            order += [("add", 6), ("add", 7)]
            order += [("exit", dg) for dg in range(24, 32)]
            for kind, i in order:
                if kind == "add":
                    s = i
                    if s == 0:
                        vector.wait_ge(sIN, NIN)
                    if s >= 2:
                        vector.wait_ge(sTANH, 2 * (s - 1))   # arg slot s%2 free
                    in0 = AP(q4, 0, [[QPC, 128], [0, DGS], [1, CH]])
                    in1 = AP(K4, 4 * DGS * s, [[NKEY + 4, 128], [4, DGS], [1, CH]])
                    outa = AP(arg[s % 2], 0, [[DGS * CH, 128], [CH, DGS], [1, CH]])
                    vector.tensor_add(outa, in0, in1).then_inc(sADD, 1)
                else:
                    dg = i
                    vector.wait_ge(sMM, dg + 1)
                    vector.tensor_copy(st32[:, CH * dg:CH * (dg + 1)],
                                       sp[dg % 4][:, :]).then_inc(sXD, 1)
            # shear merge copies
            for b in range(7):
                sh = 1 << b
                nhi = CH // (2 * sh)
                cur, nxt = cshear[b], cshear[b + 1]
                if b == 0:
                    vector.wait_ge(sEXP, 2)
                elif b >= 2:
                    vector.wait_ge(sSMM, b - 1)        # nxt slot free of PE reader
                vector.tensor_copy(AP(nxt, 0, [[CH, 128], [2 * sh, nhi], [1, sh]]),
                                   AP(cur, 0, [[CH, 128], [2 * sh, nhi], [1, sh]])
                                   ).then_inc(sSCP, 1)
                vector.wait_ge(sSMM, b + 1)
                vector.tensor_copy(AP(nxt, sh, [[CH, 128], [2 * sh, nhi], [1, sh]]),
                                   AP(rp[b % 2], 0, [[CH // 2, 128], [sh, nhi], [1, sh]])
                                   ).then_inc(sSCP, 1)
            # epilogue
            for t in range(4):
                vector.wait_ge(sVMM, t + 1)
                vector.tensor_copy(ov[t][:, :], vp[t % 2][:, :]).then_inc(sEPI, 1)

        @block.scalar
        def _(scalar):
            order = [("tanh", 0), ("tanh", 1), ("exit", 0), ("exit", 1),
                     ("exit", 2), ("exit", 3), ("tanh", 2), ("exit", 4),
                     ("exit", 5), ("exit", 6), ("exit", 7), ("tanh", 3),
                     ("tanh", 4), ("tanh", 5)]
            order += [("exit", dg) for dg in range(16, 20)]
            order.append(("tanh", 6))
            order += [("exit", dg) for dg in range(20, 24)]
            order.append(("tanh", 7))
            for kind, i in order:
                if kind == "tanh":
                    s = i
                    scalar.wait_ge(sADD, s + 1)
                    if s >= 2:
                        scalar.wait_ge(sMM, 4 * (s - 1))   # th slot s%2 free
                    scalar.activation(th[s % 2][:, :], arg[s % 2][:, :],
                                      Tanh).then_inc(sTANH, 2)
                else:
                    dg = i
                    scalar.wait_ge(sMM, dg + 1)
                    scalar.activation(st32[:, CH * dg:CH * (dg + 1)],
                                      sp[dg % 4][:, :], Tanh,
                                      bias=bat[0:4, 0:1],
                                      scale=0.5).then_inc(sXA, 1)
            scalar.wait_ge(sPL, 16 * 16)
            scalar.activation(sg[32:64, :], sg[32:64, :], Tanh,
                              bias=bat[32:64, 0:1], scale=0.5)
            scalar.wait_ge(sPL, 16 * 32)
            scalar.activation(sg[96:128, :], sg[96:128, :], Tanh,
                              bias=bat[96:128, 0:1], scale=0.5).then_inc(sEXP, 1)
            scalar.activation(cshear[0][:, :], sg[:, :], Exp,
                              bias=bat[:, 1:2], scale=0.5).then_inc(sEXP, 1)

        @block.tensor
        def _(tensor):
            tensor.wait_ge(sIN, NIN)
            for dg in range(32):
                s = dg // DGS
                tensor.wait_ge(sTANH, 2 * s + (1 if dg % DGS < 2 else 2))
                if dg >= 4:              # sp slot dg%4 free after exit dg-4
                    p = dg - 4
                    if p < 8:
                        tensor.wait_ge(sXA, p + 1)
                    elif p < 16:
                        tensor.wait_ge(sXD, p - 7)
                    elif p < 24:
                        tensor.wait_ge(sXA, p - 7)
                    else:
                        tensor.wait_ge(sXD, p - 15)
                dgl = dg % DGS
                tensor.matmul(sp[dg % 4][:, :], W4[:, :],
                              th[s % 2][:, CH * dgl:CH * (dgl + 1)],
                              start=True, stop=True).then_inc(sMM, 1)
            for b in range(7):
                sh = 1 << b
                nhi = CH // (2 * sh)
                cur = cshear[b]
                if b == 0:
                    tensor.wait_ge(sEXP, 2)
                else:
                    tensor.wait_ge(sSCP, 2 * b)
                tensor.matmul(AP(rp[b % 2], 0, [[CH // 2, 128], [sh, nhi], [1, sh]]),
                              P7[:, 128 * b:128 * (b + 1)],
                              AP(cur, sh, [[CH, 128], [2 * sh, nhi], [1, sh]]),
                              start=True, stop=True).then_inc(sSMM, 1)
            for t in range(4):
                tensor.wait_ge(sMSK, 2 * t + 1)
                if t >= 2:
                    tensor.wait_ge(sEPI, t - 1)    # vp slot t%2 free
                tensor.matmul(vp[t % 2][:, :], El[t][:, :], X[t][:, :],
                              start=True, stop=False)
                tensor.wait_ge(sMSK, 2 * t + 2)
                tensor.matmul(vp[t % 2][:, :], Eh[t][:, :], X[t + 1][:, :],
                              start=False, stop=True).then_inc(sVMM, 1)

        @block.gpsimd
        def _(gpsimd):
            gpsimd.wait_ge(sSCP, 14)
            cfin = cshear[7]
            for t in range(4):
                sl = cfin[:, 128 * t:128 * (t + 1)]
                gpsimd.affine_select(
                    El[t][:, :], sl, pattern=[[-1, 128]],
                    compare_op=mybir.AluOpType.is_ge, fill=0.0,
                    base=0, channel_multiplier=1).then_inc(sMSK, 1)
                gpsimd.affine_select(
                    Eh[t][:, :], sl, pattern=[[1, 128]],
                    compare_op=mybir.AluOpType.is_ge, fill=0.0,
                    base=-1, channel_multiplier=-1).then_inc(sMSK, 1)
            gpsimd.wait_ge(sOUT, 64)

    nc.finalize()
    return nc


def _prep_inputs(x, mask, Wt, Wx, bh, Wa, ba):
    """Build the 8 per-core input maps (host-side sharding + projections)."""
    x64 = x.astype(np.float64)
    P7 = np.zeros((128, 128 * 7), np.float32)
    for b in range(7):
        sh = 1 << b
        m = np.arange(128)
        P7[(m - sh) % 128, 128 * b + m] = 1.0
    P7 = P7.astype(BF16)
    in_maps = []
    for c in range(NCORES):
        b = c // 4
        qs = (c % 4) * QPC
        q = (x64[b] @ Wt.astype(np.float64) + bh.astype(np.float64))
        k = (x64[b] @ Wx.astype(np.float64))
        qT = q[qs:qs + QPC].T.astype(np.float32)          # [32, 512]
        q4 = np.tile(qT, (4, 1)).astype(BF16)             # [128, 512]
        kx = np.zeros((NKEY + 8, U), np.float64)
        lo = qs - 64
        s0, s1 = max(0, lo), min(L, lo + NKEY)
        kx[s0 - lo:s1 - lo] = k[s0:s1]
        K4 = np.zeros((128, NKEY + 4), np.float32)
        for dm in range(4):
            K4[32 * dm:32 * (dm + 1), :] = kx[dm:dm + NKEY + 4].T
        K4 = K4.astype(BF16)
        W4 = np.zeros((128, 4), np.float32)
        for dm in range(4):
            W4[32 * dm:32 * (dm + 1), dm] = Wa[:, 0]
        W4 = W4.astype(BF16)
        Xe = np.zeros((NKEY, 132), np.float32)
        mk = mask[b].astype(np.float32)
        xr = np.zeros((NKEY, F), np.float32)
        xr[s0 - lo:s1 - lo] = x[b, s0:s1] * mk[s0:s1, None]
        Xe[:, :F] = xr
        val = np.zeros(NKEY, np.float32)
        val[s0 - lo:s1 - lo] = mk[s0:s1]
        Xe[:, F] = val
        Xe = Xe.astype(BF16)
        bat = np.zeros((128, 2), np.float32)
        bat[:, 0] = 0.5 * float(ba[0])
        bat[:, 1] = 0.5
        in_maps.append({"q4": q4, "K4": K4, "W4": W4, "P7": P7,
                        "Xe": Xe, "bat": bat})
    return in_maps


def kernel(x, mask, Wt, Wx, bh, Wa, ba, _want_results=False):
    global _built
    from concourse.bass_utils import run_bass_kernel_spmd
    x = np.asarray(x)
    mask = np.asarray(mask)
    Wt, Wx, bh, Wa, ba = (np.asarray(a) for a in (Wt, Wx, bh, Wa, ba))
    if _built is None:
        _built = _build()
    nc = _built
    in_maps = _prep_inputs(x, mask, Wt, Wx, bh, Wa, ba)
    res = run_bass_kernel_spmd(nc, in_maps, core_ids=list(range(NCORES)))
    v = np.zeros((B, L, F), np.float32)
    for c in range(NCORES):
        b = c // 4
        qs = (c % 4) * QPC
        o = res.results[c]["out"]
        v[b, qs:qs + QPC] = o[:, :F] / (o[:, F:F + 1] + EPS)
    v *= mask.astype(np.float32)[:, :, None]
    if _want_results:
        return v, res
    return v



# revision 7
# speedup vs baseline: 1.4964x; 1.4964x over previous
"""Banded additive attention (width-128) on 8 TRN2 NeuronCores — raw Bass.

Problem: B=2, L=2048, F=128, U=32, WIDTH=128
  q = x@Wt + bh, k = x@Wx                       (host, like baseline)
  s_ij = Wa . tanh(q_i + k_j)                   (j in [i-64, i+63])
  e_ij = exp(sigmoid(s_ij)) * band * mask
  v_i  = sum_j e_ij x_j / (sum_j e_ij + 1e-7)

Sharding: core c handles batch c//4, queries [(c%4)*512, +512).  No
collectives.  Raw Bass; standalone wait_ge with hand-computed thresholds.

v2 pipeline (query-chunked; chunk widths SIZES, all mult of 16):
  DVE  : add   arg[p=(dm,u), (dg,il)] = q4[p, il] + K4[p, 4*dg + il]
  ACT  : tanh  in-place on arg (the bulk: 512*128*32 elems/core)
  PE   : 32 accumulating matmuls per chunk into ONE [128,512] psum bank:
         mm(ds,g): lhsT = W32g[:, 32g:+32] (block-diag Wa), rhs = dg-block,
         out rows 32ds..+32 (tile_position col = 32ds), d = 32ds+4g+dm
  ACT  : sig = Sigmoid(psum + ba)  (single full-height psum exit)
  ACT  : e = Exp(sig)
  PE   : radix-4 shear (3 levels: rot {0..3}, {0,4,8,12}, {16k}) via
         permutation matmuls on col classes; DVE merge copy per level
  Pool : affine_select triangles -> El/Eh per 128-query block
  PE   : v = El.T @ X[t] + Eh.T @ X[t+1]  (X has validity col -> denom)
  DVE  : ov = vp;  DMA out.  Host divides by denom and applies mask.
"""

import numpy as np
import ml_dtypes

B, L, F, U = 2, 2048, 128, 32
WIDTH = 128
EPS = 1e-7
NCORES = 8
QPC = (B * L) // NCORES          # 512 queries per core
NKEY = QPC + WIDTH               # 640 key rows per core
NK4 = NKEY + 4                   # K4 row pitch
BF16 = ml_dtypes.bfloat16

SIZES = [64, 112, 112, 112, 80, 32]      # query chunks (mult of 16)
C0S = [0, 64, 176, 288, 400, 480]
NCH = len(SIZES)
ROTS = [0, 1, 2, 3, 4, 8, 12, 16, 32, 48, 64, 80, 96, 112]
RIDX = {r: i for i, r in enumerate(ROTS)}
AMAX = max(SIZES) * U            # 3584: arg buffer cols

# (block, global lo, global hi) select pieces per chunk; block t covers
# cols [128t, 128t+128)
PIECES = []
for c in range(NCH):
    lo, hi = C0S[c], C0S[c] + SIZES[c]
    ps = []
    for t in range(4):
        a, b = max(lo, 128 * t), min(hi, 128 * (t + 1))
        if a < b:
            ps.append((t, a, b))
    PIECES.append(ps)
# chunk after which block t's selects are complete
LASTCH = {}
for c in range(NCH):
    for (t, a, b) in PIECES[c]:
        LASTCH[t] = c
# sMSK threshold per block (2 incs per piece, emission order)
_msk = 0
MSKTHR = {}
for c in range(NCH):
    for (t, a, b) in PIECES[c]:
        _msk += 2
        MSKTHR[t] = _msk

_built = None


def _build():
    import concourse.bass as bass
    import concourse.mybir as mybir

    f32 = mybir.dt.float32
    bf16 = mybir.dt.bfloat16
    Sig = mybir.ActivationFunctionType.Sigmoid
    Exp = mybir.ActivationFunctionType.Exp
    Tanh = mybir.ActivationFunctionType.Tanh
    AP = bass.AP

    nc = bass.Bass()

    q4_d = nc.dram_tensor("q4", [128, QPC], bf16, kind="ExternalInput")
    K4_d = nc.dram_tensor("K4", [128, NK4], bf16, kind="ExternalInput")
    W_d = nc.dram_tensor("W32g", [128, 256], bf16, kind="ExternalInput")
    R_d = nc.dram_tensor("Rm", [128, 128 * len(ROTS)], bf16,
                         kind="ExternalInput")
    Xe_d = nc.dram_tensor("Xe", [NKEY, 132], bf16, kind="ExternalInput")
    ba_d = nc.dram_tensor("bat", [128, 2], f32, kind="ExternalInput")
    out_d = nc.dram_tensor("out", [QPC, 132], f32, kind="ExternalOutput")

    al = nc.alloc_sbuf_tensor
    q4 = al("q4s", [128, QPC], bf16)
    K4 = al("K4s", [128, NK4], bf16)
    W32 = al("W32s", [128, 256], bf16)
    Rm = al("Rms", [128, 128 * len(ROTS)], bf16)
    X5 = al("X5s", [128, 660], bf16)
    bat = al("bats", [128, 2], f32)
    arg = [al(f"arg{i}", [128, AMAX], bf16) for i in range(2)]
    sig = al("sigs", [128, QPC], bf16)
    ee = al("ees", [128, QPC], bf16)
    M0 = al("M0s", [128, QPC], bf16)
    M1 = al("M1s", [128, QPC], bf16)
    CC = al("CCs", [128, QPC], bf16)
    El = [al(f"el{t}", [128, 128], bf16) for t in range(4)]
    Eh = [al(f"eh{t}", [128, 128], bf16) for t in range(4)]
    ov = [al(f"ov{t}", [128, 132], f32) for t in range(4)]

    ap_ = nc.alloc_psum_tensor
    psS = [ap_(f"psS{i}", [128, 128], f32) for i in range(3)]
    psA = ap_("psA", [128, QPC], f32)
    psB = ap_("psB", [128, QPC], f32)
    vp = [ap_(f"vp{i}", [128, 132], f32) for i in range(2)]

    sem = nc.alloc_semaphore
    sINA, sINB, sINC, sIND, sINE, sINF = (sem(n) for n in
        ("sINA", "sINB", "sINC", "sIND", "sINE", "sINF"))
    sADD, sTANH, sMM, sSIG, sEXP = (sem(n) for n in
        ("sADD", "sTANH", "sMM", "sSIG", "sEXP"))
    sSH, sSCP, sMSK, sVMM, sEPI, sOUT = (sem(n) for n in
        ("sSH", "sSCP", "sMSK", "sVMM", "sEPI", "sOUT"))


    with nc.Block() as block:

        @block.sync
        def _(sync):
            sync.dma_start(q4[:, :], q4_d[:, :]).then_inc(sINA, 16)
            sync.dma_start(K4[:, 0:192], K4_d[:, 0:192]).then_inc(sINA, 16)
            sync.dma_start(W32[:, :], W_d[:, :]).then_inc(sINB, 16)
            sync.dma_start(K4[:, 192:NK4], K4_d[:, 192:NK4]).then_inc(sINC, 16)
            sync.dma_start(bat[:, :], ba_d[:, :]).then_inc(sIND, 16)
            sync.dma_start(AP(X5, 0, [[660, 128], [132, 5], [1, 132]]),
                           AP(Xe_d, 0, [[132, 128], [128 * 132, 5], [1, 132]])
                           ).then_inc(sINE, 16)
            sync.dma_start(Rm[:, :], R_d[:, :]).then_inc(sINF, 16)
            for t in range(4):
                sync.wait_ge(sEPI, t + 1)
                sync.dma_start(out_d[128 * t:128 * (t + 1), :],
                               ov[t][:, :]).then_inc(sOUT, 16)

        @block.vector
        def _(vector):
            def add(c):
                c0, W = C0S[c], SIZES[c]
                if c == 0:
                    vector.wait_ge(sINA, 32)
                elif c == 1:
                    vector.wait_ge(sINC, 16)
                else:
                    vector.wait_ge(sMM, c - 1)       # arg slot c%2 free
                a = arg[c % 2]
                vector.tensor_add(
                    AP(a, 0, [[AMAX, 128], [W, U], [1, W]]),
                    AP(q4, c0, [[QPC, 128], [0, U], [1, W]]),
                    AP(K4, c0, [[NK4, 128], [4, U], [1, W]])
                ).then_inc(sADD, 1)

            def copies(c):
                c0, W = C0S[c], SIZES[c]
                for li, (dst, src) in enumerate([(M0, psA), (M1, psB),
                                                 (CC, psA)]):
                    vector.wait_ge(sSH, 3 * c + li + 1)
                    vector.tensor_copy(dst[:, c0:c0 + W],
                                       src[:, c0:c0 + W]).then_inc(sSCP, 1)

            def epi(t):
                vector.wait_ge(sVMM, t + 1)
                vector.tensor_copy(ov[t][:, :], vp[t % 2][:, :]
                                   ).then_inc(sEPI, 1)

            add(0); add(1); add(2)
            copies(0)
            add(3)
            copies(1)
            epi(0)
            add(4)
            copies(2)
            epi(1)
            add(5)
            copies(3)
            epi(2)
            copies(4)
            copies(5)
            epi(3)

        @block.scalar
        def _(scalar):
            def tanh(c):
                W = SIZES[c]
                scalar.wait_ge(sADD, c + 1)
                a = arg[c % 2]
                scalar.activation(AP(a, 0, [[AMAX, 128], [1, U * W]]),
                                  AP(a, 0, [[AMAX, 128], [1, U * W]]),
                                  Tanh).then_inc(sTANH, 1)

            def exitexp(c):
                c0, W = C0S[c], SIZES[c]
                if c == 0:
                    scalar.wait_ge(sIND, 16)
                scalar.wait_ge(sMM, c + 1)
                scalar.activation(sig[:, c0:c0 + W], psS[c % 3][:, 0:W],
                                  Sig, bias=bat[:, 0:1]).then_inc(sSIG, 1)
                scalar.wait_ge(sSIG, c + 1)
                scalar.activation(ee[:, c0:c0 + W], sig[:, c0:c0 + W],
                                  Exp).then_inc(sEXP, 1)

            tanh(0); tanh(1)
            for c in range(NCH):
                exitexp(c)
                if c + 2 < NCH:
                    tanh(c + 2)

        @block.tensor
        def _(tensor):
            def score(c):
                c0, W = C0S[c], SIZES[c]
                tensor.wait_ge(sTANH, c + 1)
                if c == 0:
                    tensor.wait_ge(sINB, 16)
                if c >= 3:
                    tensor.wait_ge(sSIG, c - 2)      # psS[c%3] free
                a = arg[c % 2]
                for ds in range(4):
                    for g in range(8):
                        mm = tensor.matmul(
                            psS[c % 3][32 * ds:32 * (ds + 1), 0:W],
                            W32[:, 32 * g:32 * (g + 1)],
                            AP(a, (8 * ds + g) * W, [[AMAX, 128], [1, W]]),
                            start=(g == 0), stop=(g == 7),
                            tile_position=(0, 32 * ds))
                        if ds == 3 and g == 7:
                            mm.then_inc(sMM, 1)

            def shear(c):
                c0, W = C0S[c], SIZES[c]
                # L0: rot b on cols c0+b::4, e -> psA
                tensor.wait_ge(sEXP, c + 1)
                if c == 0:
                    tensor.wait_ge(sINF, 16)
                else:
                    tensor.wait_ge(sSCP, 3 * c)      # psA free (CC copy c-1)
                for b in range(4):
                    r = RIDX[b]
                    mm = tensor.matmul(
                        psA[:, c0 + b * (W // 4):c0 + (b + 1) * (W // 4)],
                        Rm[:, 128 * r:128 * (r + 1)],
                        AP(ee, c0 + b, [[QPC, 128], [4, W // 4]]),
                        start=True, stop=True)
                    if b == 3:
                        mm.then_inc(sSH, 1)
                # L1: rot 4*b1 on class (il//4)%4==b1 (compacted), M0 -> psB
                tensor.wait_ge(sSCP, 3 * c + 1)
                for b1 in range(4):
                    r = RIDX[4 * b1]
                    mm = tensor.matmul(
                        psB[:, c0 + b1 * (W // 4):c0 + (b1 + 1) * (W // 4)],
                        Rm[:, 128 * r:128 * (r + 1)],
                        AP(M0, c0 + b1,
                           [[QPC, 128], [W // 4, 4], [4, W // 16]]),
                        start=True, stop=True)
                    if b1 == 3:
                        mm.then_inc(sSH, 1)
                # L2: rot 16*((c0/16 + a) % 8), class il//16 (compacted),
                # M1 -> psA; compaction digits recombine to natural order
                tensor.wait_ge(sSCP, 3 * c + 2)
                na = W // 16
                for a_ in range(na):
                    r = RIDX[16 * ((c0 // 16 + a_) % 8)]
                    mm = tensor.matmul(
                        psA[:, c0 + 16 * a_:c0 + 16 * (a_ + 1)],
                        Rm[:, 128 * r:128 * (r + 1)],
                        AP(M1, c0 + a_, [[QPC, 128], [W // 16, 16]]),
                        start=True, stop=True)
                    if a_ == na - 1:
                        mm.then_inc(sSH, 1)

            def value(t):
                tensor.wait_ge(sMSK, MSKTHR[t])
                if t == 0:
                    tensor.wait_ge(sINE, 16)
                if t >= 2:
                    tensor.wait_ge(sEPI, t - 1)      # vp slot t%2 free
                tensor.matmul(vp[t % 2][:, :], El[t][:, :],
                              X5[:, 132 * t:132 * t + 132],
                              start=True, stop=False)
                tensor.matmul(vp[t % 2][:, :], Eh[t][:, :],
                              X5[:, 132 * (t + 1):132 * (t + 1) + 132],
                              start=False, stop=True).then_inc(sVMM, 1)

            for c in range(NCH):
                score(c)
                if c >= 1:
                    shear(c - 1)
                    for t in range(4):
                        if LASTCH[t] == c - 1:
                            value(t)
            shear(NCH - 1)
            for t in range(4):
                if LASTCH[t] == NCH - 1:
                    value(t)

        @block.gpsimd
        def _(gpsimd):
            for c in range(NCH):
                gpsimd.wait_ge(sSCP, 3 * c + 3)
                for (t, a, b) in PIECES[c]:
                    gpsimd.affine_select(
                        El[t][:, a - 128 * t:b - 128 * t], CC[:, a:b],
                        pattern=[[-1, b - a]],
                        compare_op=mybir.AluOpType.is_ge, fill=0.0,
                        base=128 * t - a,
                        channel_multiplier=1).then_inc(sMSK, 1)
                    gpsimd.affine_select(
                        Eh[t][:, a - 128 * t:b - 128 * t], CC[:, a:b],
                        pattern=[[1, b - a]],
                        compare_op=mybir.AluOpType.is_ge, fill=0.0,
                        base=a - 128 * t - 1,
                        channel_multiplier=-1).then_inc(sMSK, 1)
            gpsimd.wait_ge(sOUT, 64)

    nc.finalize()
    return nc


def _prep_inputs(x, mask, Wt, Wx, bh, Wa, ba):
    """Build the 8 per-core input maps (host-side sharding + projections)."""
    x64 = x.astype(np.float64)
    # permutation/rotation matrices: R_r[(c - r) % 128, c] = 1
    Rm = np.zeros((128, 128 * len(ROTS)), np.float32)
    cix = np.arange(128)
    for i, r in enumerate(ROTS):
        Rm[(cix - r) % 128, 128 * i + cix] = 1.0
    Rm = Rm.astype(BF16)
    # W32g: mm g maps rows (dm,u) -> col 4g+dm with weight Wa[u]
    W32 = np.zeros((128, 256), np.float32)
    for g in range(8):
        for dm in range(4):
            W32[32 * dm:32 * (dm + 1), 32 * g + 4 * g + dm] = Wa[:, 0]
    W32 = W32.astype(BF16)
    in_maps = []
    for c in range(NCORES):
        b = c // 4
        qs = (c % 4) * QPC
        q = (x64[b] @ Wt.astype(np.float64) + bh.astype(np.float64))
        k = (x64[b] @ Wx.astype(np.float64))
        qT = q[qs:qs + QPC].T.astype(np.float32)          # [32, 512]
        q4 = np.tile(qT, (4, 1)).astype(BF16)             # [128, 512]
        kx = np.zeros((NKEY + 8, U), np.float64)
        lo = qs - 64
        s0, s1 = max(0, lo), min(L, lo + NKEY)
        kx[s0 - lo:s1 - lo] = k[s0:s1]
        K4 = np.zeros((128, NK4), np.float32)
        for dm in range(4):
            K4[32 * dm:32 * (dm + 1), :] = kx[dm:dm + NK4].T
        K4 = K4.astype(BF16)
        Xe = np.zeros((NKEY, 132), np.float32)
        mk = mask[b].astype(np.float32)
        xr = np.zeros((NKEY, F), np.float32)
        xr[s0 - lo:s1 - lo] = x[b, s0:s1] * mk[s0:s1, None]
        Xe[:, :F] = xr
        val = np.zeros(NKEY, np.float32)
        val[s0 - lo:s1 - lo] = mk[s0:s1]
        Xe[:, F] = val
        Xe = Xe.astype(BF16)
        bat = np.zeros((128, 2), np.float32)
        bat[:, 0] = float(ba[0])
        in_maps.append({"q4": q4, "K4": K4, "W32g": W32, "Rm": Rm,
                        "Xe": Xe, "bat": bat})
    return in_maps


def kernel(x, mask, Wt, Wx, bh, Wa, ba, _want_results=False):
    global _built
    from concourse.bass_utils import run_bass_kernel_spmd
    x = np.asarray(x)
    mask = np.asarray(mask)
    Wt, Wx, bh, Wa, ba = (np.asarray(a) for a in (Wt, Wx, bh, Wa, ba))
    if _built is None:
        _built = _build()
    nc = _built
    in_maps = _prep_inputs(x, mask, Wt, Wx, bh, Wa, ba)
    res = run_bass_kernel_spmd(nc, in_maps, core_ids=list(range(NCORES)))
    v = np.zeros((B, L, F), np.float32)
    for c in range(NCORES):
        b = c // 4
        qs = (c % 4) * QPC
        o = res.results[c]["out"]
        v[b, qs:qs + QPC] = o[:, :F] / (o[:, F:F + 1] + EPS)
    v *= mask.astype(np.float32)[:, :, None]
    if _want_results:
        return v, res
    return v


# revision 10
# speedup vs baseline: 1.7487x; 1.1685x over previous
"""Banded additive attention (width-128) on 8 TRN2 NeuronCores — raw Bass.

Problem: B=2, L=2048, F=128, U=32, WIDTH=128
  q = x@Wt + bh, k = x@Wx                       (host, like baseline)
  s_ij = Wa . tanh(q_i + k_j)                   (j in [i-64, i+63])
  e_ij = exp(sigmoid(s_ij)) * band * mask
  v_i  = sum_j e_ij x_j / (sum_j e_ij + 1e-7)

Sharding: core c handles batch c//4, queries [(c%4)*512, +512).  No
collectives.  Raw Bass; standalone wait_ge with hand-computed thresholds.

v3 pipeline (query-chunked; chunk widths SIZES, all mult of 16):
  DVE  : add   arg[p=(dm,u), (dg,il)] = q4[p, il] + K4[p, 4*dg + il]
         (3 arg slots so adds run 2 chunks ahead)
  ACT  : tanh  in-place on arg (the bulk: 512*128*32 elems/core)
  PE   : 32 accumulating matmuls per chunk into psS[c%3] psum bank:
         mm(ds,g): lhsT = W32g[:, 32g:+32] (block-diag Wa), rhs = dg-block,
         out rows 32ds..+32 (tile_position col = 32ds), d = 32ds+4g+dm
  ACT  : sig = Sigmoid(psum + ba); e = Exp(sig)
  PE   : 2-level shear (radix 8x16): rotation matmuls on col classes with
         compacted outputs (digits recombine to natural order):
         L-lo: rot lo=il%8 on stride-8 classes, e -> psA
         L-hi: rot 8*((c0/8 + hi)%16) on stride-(W/8) classes, M0 -> psB
  DVE  : M0 = copy(psA);  El/Eh = psB * tri-masks (replaces affine_select)
  PE   : v = El.T @ X[t] + Eh.T @ X[t+1]  (X has validity col -> denom)
  DVE  : ov = vp;  DMA out.  Host divides by denom and applies mask.
"""

import numpy as np
import ml_dtypes

B, L, F, U = 2, 2048, 128, 32
WIDTH = 128
EPS = 1e-7
NCORES = 8
QPC = (B * L) // NCORES          # 512 queries per core
NKEY = QPC + WIDTH               # 640 key rows per core
NK4 = NKEY + 4                   # K4 row pitch
BF16 = ml_dtypes.bfloat16

SIZES = [64, 112, 112, 112, 80, 32]      # query chunks (mult of 16)
C0S = [0, 64, 176, 288, 400, 480]
NCH = len(SIZES)
ROTS = [0, 1, 2, 3, 4, 5, 6, 7] + [8 * k for k in range(1, 16)]
RIDX = {r: i for i, r in enumerate(ROTS)}
NR = len(ROTS)                   # 23 rotation matrices
AMAX = max(SIZES) * U            # 3584: arg buffer cols

# (block, global lo, global hi) pieces per chunk; block t = cols [128t,+128)
PIECES = []
for c in range(NCH):
    lo, hi = C0S[c], C0S[c] + SIZES[c]
    ps = []
    for t in range(4):
        a, b = max(lo, 128 * t), min(hi, 128 * (t + 1))
        if a < b:
            ps.append((t, a, b))
    PIECES.append(ps)
LASTCH = {}                      # chunk completing block t's El/Eh
for c in range(NCH):
    for (t, a, b) in PIECES[c]:
        LASTCH[t] = c
_msk = 0
MSKTHR = {}                      # sMSK threshold per block (2 per piece)
for c in range(NCH):
    for (t, a, b) in PIECES[c]:
        _msk += 2
        MSKTHR[t] = _msk

_built = None


def _build():
    import concourse.bass as bass
    import concourse.mybir as mybir

    f32 = mybir.dt.float32
    bf16 = mybir.dt.bfloat16
    Sig = mybir.ActivationFunctionType.Sigmoid
    Exp = mybir.ActivationFunctionType.Exp
    Tanh = mybir.ActivationFunctionType.Tanh
    Mult = mybir.AluOpType.mult
    AP = bass.AP

    nc = bass.Bass()

    q4_d = nc.dram_tensor("q4", [128, QPC], bf16, kind="ExternalInput")
    K4_d = nc.dram_tensor("K4", [128, NK4], bf16, kind="ExternalInput")
    W_d = nc.dram_tensor("W32g", [128, 256], bf16, kind="ExternalInput")
    # Rm: 23 rotation matrices then maskl/maskh [128, 256]
    R_d = nc.dram_tensor("Rm", [128, 128 * NR + 256], bf16,
                         kind="ExternalInput")
    Xe_d = nc.dram_tensor("Xe", [NKEY, 132], bf16, kind="ExternalInput")
    ba_d = nc.dram_tensor("bat", [128, 2], f32, kind="ExternalInput")
    out_d = nc.dram_tensor("out", [QPC, 132], f32, kind="ExternalOutput")

    al = nc.alloc_sbuf_tensor
    q4 = al("q4s", [128, QPC], bf16)
    K4 = al("K4s", [128, NK4], bf16)
    W32 = al("W32s", [128, 256], bf16)
    Rm = al("Rms", [128, 128 * NR + 256], bf16)
    X5 = al("X5s", [128, 660], bf16)
    bat = al("bats", [128, 2], f32)
    arg = [al(f"arg{i}", [128, AMAX], bf16) for i in range(3)]
    sig = al("sigs", [128, QPC], bf16)
    ee = al("ees", [128, QPC], bf16)
    M0 = al("M0s", [128, QPC], bf16)
    El = [al(f"el{t}", [128, 128], bf16) for t in range(4)]
    Eh = [al(f"eh{t}", [128, 128], bf16) for t in range(4)]
    ov = [al(f"ov{t}", [128, 132], f32) for t in range(4)]
    MKL = 128 * NR               # maskl col offset in Rm
    MKH = 128 * NR + 128

    ap_ = nc.alloc_psum_tensor
    psS = [ap_(f"psS{i}", [128, 128], f32) for i in range(3)]
    psA = ap_("psA", [128, QPC], f32)
    psB = ap_("psB", [128, QPC], f32)
    vp = [ap_(f"vp{i}", [128, 132], f32) for i in range(2)]

    sem = nc.alloc_semaphore
    sINA, sINB, sINC, sIND, sINE, sINF, sING = (sem(n) for n in
        ("sINA", "sINB", "sINC", "sIND", "sINE", "sINF", "sING"))
    sADD, sTANH, sMM, sSIG, sEXP = (sem(n) for n in
        ("sADD", "sTANH", "sMM", "sSIG", "sEXP"))
    sSH, sSCP, sMSK, sVMM, sEPI, sOUT = (sem(n) for n in
        ("sSH", "sSCP", "sMSK", "sVMM", "sEPI", "sOUT"))

    with nc.Block() as block:

        @block.sync
        def _(sync):
            # SP queue: q4, W32g, K4r, bat, Xe; Pool queue: K4p, Rm+masks
            sync.dma_start(q4[:, :], q4_d[:, :]).then_inc(sINA, 16)
            sync.dma_start(W32[:, :], W_d[:, :]).then_inc(sINB, 16)
            sync.dma_start(K4[:, 192:NK4], K4_d[:, 192:NK4]).then_inc(sINC, 16)
            sync.dma_start(bat[:, :], ba_d[:, :]).then_inc(sIND, 16)
            sync.dma_start(AP(X5, 0, [[660, 128], [132, 5], [1, 132]]),
                           AP(Xe_d, 0, [[132, 128], [128 * 132, 5], [1, 132]])
                           ).then_inc(sINE, 16)
            for t in range(4):
                sync.wait_ge(sEPI, t + 1)
                sync.dma_start(out_d[128 * t:128 * (t + 1), :],
                               ov[t][:, :]).then_inc(sOUT, 16)

        @block.vector
        def _(vector):
            def add(c):
                c0, W = C0S[c], SIZES[c]
                if c == 0:
                    vector.wait_ge(sINA, 16)
                    vector.wait_ge(sING, 16)
                elif c == 1:
                    vector.wait_ge(sINC, 16)
                else:
                    vector.wait_ge(sMM, c - 2)       # arg slot c%3 free
                a = arg[c % 3]
                vector.tensor_add(
                    AP(a, 0, [[AMAX, 128], [W, U], [1, W]]),
                    AP(q4, c0, [[QPC, 128], [0, U], [1, W]]),
                    AP(K4, c0, [[NK4, 128], [4, U], [1, W]])
                ).then_inc(sADD, 1)

            def m0copy(c):
                c0, W = C0S[c], SIZES[c]
                vector.wait_ge(sSH, 2 * c + 1)
                vector.tensor_copy(M0[:, c0:c0 + W],
                                   psA[:, c0:c0 + W]).then_inc(sSCP, 1)

            def eleh(c):
                vector.wait_ge(sSH, 2 * c + 2)
                for (t, a, b) in PIECES[c]:
                    la, lb = a - 128 * t, b - 128 * t
                    vector.tensor_tensor(
                        El[t][:, la:lb], psB[:, a:b],
                        Rm[:, MKL + la:MKL + lb], Mult).then_inc(sMSK, 1)
                    vector.tensor_tensor(
                        Eh[t][:, la:lb], psB[:, a:b],
                        Rm[:, MKH + la:MKH + lb], Mult).then_inc(sMSK, 1)

            def epi(t):
                vector.wait_ge(sVMM, t + 1)
                vector.tensor_copy(ov[t][:, :], vp[t % 2][:, :]
                                   ).then_inc(sEPI, 1)

            add(0); add(1); add(2)
            m0copy(0); eleh(0)
            add(3)
            m0copy(1); eleh(1)
            add(4)
            m0copy(2); eleh(2)
            epi(0)
            add(5)
            m0copy(3); eleh(3)
            epi(1)
            m0copy(4); eleh(4)
            epi(2)
            m0copy(5); eleh(5)
            epi(3)

        @block.scalar
        def _(scalar):
            def tanh(c):
                W = SIZES[c]
                scalar.wait_ge(sADD, c + 1)
                a = arg[c % 3]
                scalar.activation(AP(a, 0, [[AMAX, 128], [1, U * W]]),
                                  AP(a, 0, [[AMAX, 128], [1, U * W]]),
                                  Tanh).then_inc(sTANH, 1)

            def exitexp(c):
                c0, W = C0S[c], SIZES[c]
                if c == 0:
                    scalar.wait_ge(sIND, 16)
                scalar.wait_ge(sMM, c + 1)
                scalar.activation(sig[:, c0:c0 + W], psS[c % 3][:, 0:W],
                                  Sig, bias=bat[:, 0:1]).then_inc(sSIG, 1)
                scalar.wait_ge(sSIG, c + 1)
                scalar.activation(ee[:, c0:c0 + W], sig[:, c0:c0 + W],
                                  Exp).then_inc(sEXP, 1)

            tanh(0); tanh(1)
            for c in range(NCH):
                exitexp(c)
                if c + 2 < NCH:
                    tanh(c + 2)

        @block.tensor
        def _(tensor):
            def score(c):
                c0, W = C0S[c], SIZES[c]
                tensor.wait_ge(sTANH, c + 1)
                if c == 0:
                    tensor.wait_ge(sINB, 16)
                if c >= 3:
                    tensor.wait_ge(sSIG, c - 2)      # psS[c%3] free
                a = arg[c % 3]
                for ds in range(4):
                    for g in range(8):
                        mm = tensor.matmul(
                            psS[c % 3][32 * ds:32 * (ds + 1), 0:W],
                            W32[:, 32 * g:32 * (g + 1)],
                            AP(a, (8 * ds + g) * W, [[AMAX, 128], [1, W]]),
                            start=(g == 0), stop=(g == 7),
                            tile_position=(0, 32 * ds))
                        if ds == 3 and g == 7:
                            mm.then_inc(sMM, 1)

            def shear(c):
                c0, W = C0S[c], SIZES[c]
                nlo = W // 8
                # L-lo: rot lo on cols c0+lo::8, e -> psA (compacted)
                tensor.wait_ge(sEXP, c + 1)
                if c == 0:
                    tensor.wait_ge(sINF, 16)
                for lo in range(8):
                    r = RIDX[lo]
                    mm = tensor.matmul(
                        psA[:, c0 + lo * nlo:c0 + (lo + 1) * nlo],
                        Rm[:, 128 * r:128 * (r + 1)],
                        AP(ee, c0 + lo, [[QPC, 128], [8, nlo]]),
                        start=True, stop=True)
                    if lo == 7:
                        mm.then_inc(sSH, 1)
                # L-hi: rot 8*((c0/8 + hi)%16) on stride-nlo classes,
                # M0 -> psB; compaction digits recombine to natural order
                tensor.wait_ge(sSCP, c + 1)
                for hi in range(nlo):
                    r = RIDX[8 * ((c0 // 8 + hi) % 16)]
                    mm = tensor.matmul(
                        psB[:, c0 + 8 * hi:c0 + 8 * (hi + 1)],
                        Rm[:, 128 * r:128 * (r + 1)],
                        AP(M0, c0 + hi, [[QPC, 128], [nlo, 8]]),
                        start=True, stop=True)
                    if hi == nlo - 1:
                        mm.then_inc(sSH, 1)

            def value(t):
                tensor.wait_ge(sMSK, MSKTHR[t])
                if t == 0:
                    tensor.wait_ge(sINE, 16)
                if t >= 2:
                    tensor.wait_ge(sEPI, t - 1)      # vp slot t%2 free
                tensor.matmul(vp[t % 2][:, :], El[t][:, :],
                              X5[:, 132 * t:132 * t + 132],
                              start=True, stop=False)
                tensor.matmul(vp[t % 2][:, :], Eh[t][:, :],
                              X5[:, 132 * (t + 1):132 * (t + 1) + 132],
                              start=False, stop=True).then_inc(sVMM, 1)

            for c in range(NCH):
                score(c)
                if c >= 1:
                    shear(c - 1)
                for t in range(4):
                    if LASTCH[t] == c - 2:
                        value(t)
            shear(NCH - 1)
            for t in range(4):
                if LASTCH[t] >= NCH - 2:
                    value(t)

        @block.gpsimd
        def _(gpsimd):
            gpsimd.dma_start(K4[:, 0:192], K4_d[:, 0:192]).then_inc(sING, 16)
            gpsimd.dma_start(Rm[:, :], R_d[:, :]).then_inc(sINF, 16)
            gpsimd.wait_ge(sOUT, 64)

    nc.finalize()
    return nc


def _prep_inputs(x, mask, Wt, Wx, bh, Wa, ba):
    """Build the 8 per-core input maps (host-side sharding + projections)."""
    x64 = x.astype(np.float64)
    # rotation matrices R_r[(c - r) % 128, c] = 1, then tri masks
    Rm = np.zeros((128, 128 * NR + 256), np.float32)
    cix = np.arange(128)
    for i, r in enumerate(ROTS):
        Rm[(cix - r) % 128, 128 * i + cix] = 1.0
    cc = cix[:, None]
    il = cix[None, :]
    Rm[:, 128 * NR:128 * NR + 128] = (cc >= il).astype(np.float32)
    Rm[:, 128 * NR + 128:] = (cc < il).astype(np.float32)
    Rm = Rm.astype(BF16)
    # W32g: mm g maps rows (dm,u) -> col 4g+dm with weight Wa[u]
    W32 = np.zeros((128, 256), np.float32)
    for g in range(8):
        for dm in range(4):
            W32[32 * dm:32 * (dm + 1), 32 * g + 4 * g + dm] = Wa[:, 0]
    W32 = W32.astype(BF16)
    in_maps = []
    for c in range(NCORES):
        b = c // 4
        qs = (c % 4) * QPC
        q = (x64[b] @ Wt.astype(np.float64) + bh.astype(np.float64))
        k = (x64[b] @ Wx.astype(np.float64))
        qT = q[qs:qs + QPC].T.astype(np.float32)          # [32, 512]
        q4 = np.tile(qT, (4, 1)).astype(BF16)             # [128, 512]
        kx = np.zeros((NKEY + 8, U), np.float64)
        lo = qs - 64
        s0, s1 = max(0, lo), min(L, lo + NKEY)
        kx[s0 - lo:s1 - lo] = k[s0:s1]
        K4 = np.zeros((128, NK4), np.float32)
        for dm in range(4):
            K4[32 * dm:32 * (dm + 1), :] = kx[dm:dm + NK4].T
        K4 = K4.astype(BF16)
        Xe = np.zeros((NKEY, 132), np.float32)
        mk = mask[b].astype(np.float32)
        xr = np.zeros((NKEY, F), np.float32)
        xr[s0 - lo:s1 - lo] = x[b, s0:s1] * mk[s0:s1, None]
        Xe[:, :F] = xr
        val = np.zeros(NKEY, np.float32)
        val[s0 - lo:s1 - lo] = mk[s0:s1]
        Xe[:, F] = val
        Xe = Xe.astype(BF16)
        bat = np.zeros((128, 2), np.float32)
        bat[:, 0] = float(ba[0])
        in_maps.append({"q4": q4, "K4": K4, "W32g": W32, "Rm": Rm,
                        "Xe": Xe, "bat": bat})
    return in_maps


def kernel(x, mask, Wt, Wx, bh, Wa, ba, _want_results=False):
    global _built
    from concourse.bass_utils import run_bass_kernel_spmd
    x = np.asarray(x)
    mask = np.asarray(mask)
    Wt, Wx, bh, Wa, ba = (np.asarray(a) for a in (Wt, Wx, bh, Wa, ba))
    if _built is None:
        _built = _build()
    nc = _built
    in_maps = _prep_inputs(x, mask, Wt, Wx, bh, Wa, ba)
    res = run_bass_kernel_spmd(nc, in_maps, core_ids=list(range(NCORES)))
    v = np.zeros((B, L, F), np.float32)
    for c in range(NCORES):
        b = c // 4
        qs = (c % 4) * QPC
        o = res.results[c]["out"]
        v[b, qs:qs + QPC] = o[:, :F] / (o[:, F:F + 1] + EPS)
    v *= mask.astype(np.float32)[:, :, None]
    if _want_results:
        return v, res
    return v
